# revision 4
# baseline (speedup 1.0000x reference)
"""Trainium2 Bass kernel for nn_Attention (B=4, S=2048, D=1024, H=16) on 8 NeuronCores.

Sharding: data-parallel over (batch, sequence-half) -> 8 shards, one per core.
Each core computes attention for 1024 query tokens of one batch element:
  - K/V projections over the full 2048-token sequence of its batch element
    (duplicated across the 2 cores sharing a batch element -- cheaper than
    communicating K/V, so the kernel needs no collectives),
  - Q projection for its 1024 queries,
  - per-head S^T = K_h @ Q_h^T, softmax over keys via exp + ones-column
    denominator folded into the A@V matmul,
  - output projection + residual + LayerNorm on its 1024 tokens.

Layouts keep every matmul contraction on the partition axis:
  xt = X[b].T in bf16 (queries-first token order) [D, S]; projections produce
  Q^T, K^T (bf16) and V natural (bf16, ones column appended per head for the
  softmax denominator). All matmuls are bf16 inputs with fp32 PSUM
  accumulation; softmax statistics, residual and LayerNorm stay fp32.
"""

import os
import sys

sys.path.insert(0, "/opt/trn_rl_repo")

import numpy as np

B, S, D, H = 4, 2048, 1024, 16
HD = D // H  # 64
SQ = S // 2  # queries per core
NCORES = 8
EPS = 1e-12

_CACHE = {}


def _install_ntff_hook():
    """Register the axon NTFF profile hook that bass_utils looks up via
    antenv.axon_hooks (absent from the image's antenv stub)."""
    import contextlib
    import ctypes
    import types

    so_path = "/opt/axon/libaxon_pjrt.so"
    if "antenv.axon_hooks" in sys.modules:
        return
    try:
        lib = ctypes.CDLL(so_path)
    except OSError:
        return
    if not hasattr(lib, "axon_start_nrt_profile"):
        return
    lib.axon_start_nrt_profile.argtypes = [ctypes.POINTER(ctypes.c_int64), ctypes.c_size_t]
    lib.axon_start_nrt_profile.restype = ctypes.c_int64
    lib.axon_stop_nrt_profile.argtypes = [ctypes.c_char_p]
    lib.axon_stop_nrt_profile.restype = ctypes.c_int64

    @contextlib.contextmanager
    def _hook(output_dir, device_ids):
        import jax

        jax.devices()
        if device_ids:
            ids = (ctypes.c_int64 * len(device_ids))(*device_ids)
            rc = lib.axon_start_nrt_profile(ids, len(device_ids))
        else:
            rc = lib.axon_start_nrt_profile(None, 0)
        if rc != 0:
            raise RuntimeError(f"axon_start_nrt_profile rc={rc}")
        try:
            yield
        finally:
            n = lib.axon_stop_nrt_profile(str(output_dir).encode())
            if n < 0:
                raise RuntimeError(f"axon_stop_nrt_profile rc={n}")

    m = types.ModuleType("antenv.axon_hooks")
    m.get_axon_ntff_profile_hook = lambda: _hook
    m.set_axon_ntff_profile_hook = lambda h: None
    sys.modules["antenv.axon_hooks"] = m


def _build():
    import concourse.bass as bass
    import concourse.tile as tile
    from concourse import bacc, mybir

    f32 = mybir.dt.float32
    bf16 = mybir.dt.bfloat16
    ADD = mybir.AluOpType.add
    MULT = mybir.AluOpType.mult
    SUB = mybir.AluOpType.subtract
    Exp = mybir.ActivationFunctionType.Exp
    Sqrt = mybir.ActivationFunctionType.Sqrt

    nc = bacc.Bacc("TRN2")

    xt_d = nc.dram_tensor("xt", [D, S], bf16, kind="ExternalInput")
    xq_d = nc.dram_tensor("xq", [SQ, D], f32, kind="ExternalInput")
    wq_d = nc.dram_tensor("wqt", [D, D], bf16, kind="ExternalInput")
    wk_d = nc.dram_tensor("wkt", [D, D], bf16, kind="ExternalInput")
    wv_d = nc.dram_tensor("wvt", [D, D], bf16, kind="ExternalInput")
    wo_d = nc.dram_tensor("wot", [D, D], bf16, kind="ExternalInput")
    bq_d = nc.dram_tensor("bqt", [128, 8], f32, kind="ExternalInput")
    bk_d = nc.dram_tensor("bkt", [128, 8], f32, kind="ExternalInput")
    bv_d = nc.dram_tensor("bv", [D], f32, kind="ExternalInput")
    bo_d = nc.dram_tensor("bo", [D], f32, kind="ExternalInput")
    gamma_d = nc.dram_tensor("gamma", [D], f32, kind="ExternalInput")
    beta_d = nc.dram_tensor("beta", [D], f32, kind="ExternalInput")
    sel_d = nc.dram_tensor("sel", [16, 8, 128], bf16, kind="ExternalInput")
    out_d = nc.dram_tensor("out", [SQ, D], f32, kind="ExternalOutput")

    def bcast_ap(handle):
        ap = handle[:]
        return bass.AP(tensor=ap.tensor, offset=ap.offset, ap=[[0, 128], ap.ap[0]])

    with tile.TileContext(nc) as tc:
        with (
            tc.tile_pool(name="const", bufs=1) as constp,
            tc.tile_pool(name="qt", bufs=1) as qtp,
            tc.tile_pool(name="kt", bufs=1) as ktp,
            tc.tile_pool(name="v", bufs=1) as vp,
        ):
            # --- constants ---
            bq_c = constp.tile([128, 8], f32, tag="bq")
            bk_c = constp.tile([128, 8], f32, tag="bk")
            bv_c = constp.tile([128, D], f32, tag="bv")
            bo_c = constp.tile([128, D], f32, tag="bo")
            gamma_c = constp.tile([128, D], f32, tag="gamma")
            beta_c = constp.tile([128, D], f32, tag="beta")
            eps_c = constp.tile([128, 1], f32, tag="eps")
            sel_c = constp.tile([16, 8, 128], bf16, tag="sel")
            nc.sync.dma_start(out=bq_c[:], in_=bq_d[:])
            nc.sync.dma_start(out=bk_c[:], in_=bk_d[:])
            nc.gpsimd.dma_start(out=bv_c[:], in_=bcast_ap(bv_d))
            nc.gpsimd.dma_start(out=bo_c[:], in_=bcast_ap(bo_d))
            nc.gpsimd.dma_start(out=gamma_c[:], in_=bcast_ap(gamma_d))
            nc.gpsimd.dma_start(out=beta_c[:], in_=bcast_ap(beta_d))
            nc.sync.dma_start(out=sel_c[:], in_=sel_d[:])
            nc.vector.memset(eps_c[:], EPS)

            # --- persistent activations ---
            qt = qtp.tile([128, 8, SQ], bf16, tag="qt")       # Q^T
            kt = ktp.tile([128, 8, S], bf16, tag="kt")        # K^T
            v = vp.tile([128, 16, H, HD + 1], bf16, tag="v")  # V + ones col

            nc.vector.memset(v[:, :, :, HD : HD + 1], 1.0)

            # ================= phase 1: projections =================
            with (
                tc.tile_pool(name="xt", bufs=1) as xtp,
                tc.tile_pool(name="wstr", bufs=3) as wstr,
                tc.tile_pool(name="ps1", bufs=8, space="PSUM") as ps1,
            ):
                xt = xtp.tile([128, 8, S], bf16, tag="xt")
                for r in range(8):
                    nc.sync.dma_start(out=xt[:, r, :], in_=xt_d[r * 128 : (r + 1) * 128, :])

                # Q^T projection: 1024 query tokens (first SQ columns of xt)
                for tg in range(2):
                    ps = [ps1.tile([128, 512], f32, tag="ps", name="ps") for _ in range(8)]
                    for k in range(8):
                        wt = wstr.tile([128, D], bf16, tag="w")
                        nc.sync.dma_start(out=wt[:], in_=wq_d[k * 128 : (k + 1) * 128, :])
                        for m in range(8):
                            nc.tensor.matmul(
                                out=ps[m][:],
                                lhsT=wt[:, m * 128 : (m + 1) * 128],
                                rhs=xt[:, k, tg * 512 : (tg + 1) * 512],
                                start=(k == 0),
                                stop=(k == 7),
                            )
                    for m in range(8):
                        nc.vector.tensor_scalar(
                            out=qt[:, m, tg * 512 : (tg + 1) * 512],
                            in0=ps[m][:],
                            scalar1=bq_c[:, m : m + 1],
                            scalar2=None,
                            op0=ADD,
                        )

                # K^T projection: full 2048 tokens
                for tg in range(4):
                    ps = [ps1.tile([128, 512], f32, tag="ps", name="ps") for _ in range(8)]
                    for k in range(8):
                        wt = wstr.tile([128, D], bf16, tag="w")
                        nc.sync.dma_start(out=wt[:], in_=wk_d[k * 128 : (k + 1) * 128, :])
                        for m in range(8):
                            nc.tensor.matmul(
                                out=ps[m][:],
                                lhsT=wt[:, m * 128 : (m + 1) * 128],
                                rhs=xt[:, k, tg * 512 : (tg + 1) * 512],
                                start=(k == 0),
                                stop=(k == 7),
                            )
                    for m in range(8):
                        nc.vector.tensor_scalar(
                            out=kt[:, m, tg * 512 : (tg + 1) * 512],
                            in0=ps[m][:],
                            scalar1=bk_c[:, m : m + 1],
                            scalar2=None,
                            op0=ADD,
                        )

                # V projection: V natural [tokens, D] in bf16 with ones columns
                for tcg in range(2):
                    for dg in range(2):
                        ps = [ps1.tile([128, 512], f32, tag="ps", name="ps") for _ in range(8)]
                        for k in range(8):
                            wt = wstr.tile([128, 512], bf16, tag="w")
                            nc.sync.dma_start(
                                out=wt[:],
                                in_=wv_d[k * 128 : (k + 1) * 128, dg * 512 : (dg + 1) * 512],
                            )
                            for i in range(8):
                                tc_i = tcg * 8 + i
                                nc.tensor.matmul(
                                    out=ps[i][:],
                                    lhsT=xt[:, k, tc_i * 128 : (tc_i + 1) * 128],
                                    rhs=wt[:],
                                    start=(k == 0),
                                    stop=(k == 7),
                                )
                        for i in range(8):
                            tc_i = tcg * 8 + i
                            nc.vector.tensor_tensor(
                                out=v[:, tc_i, dg * 8 : (dg + 1) * 8, 0:HD],
                                in0=ps[i][:].rearrange("p (h d) -> p h d", d=HD),
                                in1=bv_c[:, dg * 512 : (dg + 1) * 512].rearrange(
                                    "p (h d) -> p h d", d=HD
                                ),
                                op=ADD,
                            )

            # ot / den live through phases 2+3 only (SBUF headroom for phase 1)
            with (
                tc.tile_pool(name="ot", bufs=1) as otp,
                tc.tile_pool(name="den", bufs=1) as denp,
            ):
                ot = otp.tile([128, 8, SQ], bf16, tag="ot")  # O^T (unnormalized)
                den = denp.tile([16, SQ], f32, tag="den")
                recip = denp.tile([16, SQ], f32, tag="recip")
                recip_b = denp.tile([16, SQ], bf16, tag="recip_b")

                # ================= phase 2: attention =================
                with (
                    tc.tile_pool(name="st", bufs=8) as stp,
                    tc.tile_pool(name="stage", bufs=2) as stagep,
                    tc.tile_pool(name="sp", bufs=2, space="PSUM") as spp,
                    tc.tile_pool(name="av", bufs=2, space="PSUM") as avp,
                ):
                    for h in range(H):
                        p0 = (h % 2) * 64
                        m = h // 2
                        st_tiles = [
                            stp.tile([128, 4, SQ], bf16, tag="st", name="st") for _ in range(4)
                        ]
                        for kc in range(16):
                            sp = spp.tile([128, 1024], f32, tag="sp", name="sp")
                            for qh in range(2):
                                nc.tensor.matmul(
                                    out=sp[:, qh * 512 : (qh + 1) * 512],
                                    lhsT=kt[p0 : p0 + 64, m, kc * 128 : (kc + 1) * 128],
                                    rhs=qt[p0 : p0 + 64, m, qh * 512 : (qh + 1) * 512],
                                    start=True,
                                    stop=True,
                                )
                            nc.scalar.activation(
                                out=st_tiles[kc // 4][:, kc % 4, :],
                                in_=sp[:],
                                func=Exp,
                                scale=float(1.0 / np.sqrt(HD)),
                            )
                        av = avp.tile([128, 1024], f32, tag="av", name="av")
                        for qh in range(2):
                            for kc in range(16):
                                nc.tensor.matmul(
                                    out=av[0:65, qh * 512 : (qh + 1) * 512],
                                    lhsT=v[:, kc, h, :],
                                    rhs=st_tiles[kc // 4][:, kc % 4, qh * 512 : (qh + 1) * 512],
                                    start=(kc == 0),
                                    stop=(kc == 15),
                                )
                        stg_d = stagep.tile([65, SQ], f32, tag="stg_d", name="stg_d")
                        nc.vector.tensor_copy(out=stg_d[64:65, :], in_=av[64:65, :])
                        nc.sync.dma_start(out=den[h : h + 1, :], in_=stg_d[64:65, :])
                        if h % 2 == 0:
                            nc.vector.tensor_copy(out=ot[0:64, m, :], in_=av[0:64, :])
                        else:
                            stg_o = stagep.tile([64, SQ], bf16, tag="stg_o", name="stg_o")
                            nc.vector.tensor_copy(out=stg_o[:, :], in_=av[0:64, :])
                            nc.sync.dma_start(out=ot[64:128, m, :], in_=stg_o[:, :])

                    # deferred softmax normalization: ot *= 1/den (per head, query)
                    nc.vector.reciprocal(out=recip[:], in_=den[:])
                    nc.vector.tensor_copy(out=recip_b[:], in_=recip[:])
                    for m in range(8):
                        bc = avp.tile([128, 1024], f32, tag="av", name="bc")
                        for g in range(2):
                            nc.tensor.matmul(
                                out=bc[:, g * 512 : (g + 1) * 512],
                                lhsT=sel_c[:, m, :],
                                rhs=recip_b[:, g * 512 : (g + 1) * 512],
                                start=True,
                                stop=True,
                            )
                        nc.vector.tensor_tensor(
                            out=ot[:, m, :], in0=ot[:, m, :], in1=bc[:], op=MULT
                        )

                # ========== phase 3: output projection + residual + LN ==========
                with (
                    tc.tile_pool(name="wo", bufs=3) as wop,
                    tc.tile_pool(name="xqp", bufs=3) as xqp,
                    tc.tile_pool(name="y", bufs=3) as yp,
                    tc.tile_pool(name="stats", bufs=4) as statp,
                    tc.tile_pool(name="ps3", bufs=4, space="PSUM") as ps3,
                ):
                    for tg in range(2):
                        ps = [
                            ps3.tile([128, 1024], f32, tag="ps", name="ps3") for _ in range(4)
                        ]
                        for k in range(8):
                            wt = wop.tile([128, D], bf16, tag="wo")
                            nc.sync.dma_start(out=wt[:], in_=wo_d[k * 128 : (k + 1) * 128, :])
                            for i in range(4):
                                t = tg * 4 + i
                                for g in range(2):
                                    nc.tensor.matmul(
                                        out=ps[i][:, g * 512 : (g + 1) * 512],
                                        lhsT=ot[:, k, t * 128 : (t + 1) * 128],
                                        rhs=wt[:, g * 512 : (g + 1) * 512],
                                        start=(k == 0),
                                        stop=(k == 7),
                                    )
                        for i in range(4):
                            t = tg * 4 + i
                            xq_t = xqp.tile([128, D], f32, tag="xq")
                            nc.sync.dma_start(
                                out=xq_t[:], in_=xq_d[t * 128 : (t + 1) * 128, :]
                            )
                            y = yp.tile([128, D], f32, tag="y")
                            nc.vector.tensor_tensor(out=y[:], in0=ps[i][:], in1=bo_c[:], op=ADD)
                            nc.vector.tensor_tensor(out=y[:], in0=y[:], in1=xq_t[:], op=ADD)
                            stats = statp.tile([128, 2, 6], f32, tag="stats")
                            mv = statp.tile([128, 2], f32, tag="mv")
                            nc.vector.bn_stats(out=stats[:, 0, :], in_=y[:, 0:512])
                            nc.vector.bn_stats(out=stats[:, 1, :], in_=y[:, 512:1024])
                            nc.vector.bn_aggr(out=mv[:], in_=stats[:])
                            nc.scalar.activation(
                                out=mv[:, 1:2], in_=mv[:, 1:2], func=Sqrt, bias=eps_c[:, 0:1]
                            )
                            nc.vector.reciprocal(out=mv[:, 1:2], in_=mv[:, 1:2])
                            nc.vector.tensor_scalar(
                                out=y[:],
                                in0=y[:],
                                scalar1=mv[:, 0:1],
                                scalar2=mv[:, 1:2],
                                op0=SUB,
                                op1=MULT,
                            )
                            nc.vector.tensor_tensor(out=y[:], in0=y[:], in1=gamma_c[:], op=MULT)
                            nc.vector.tensor_tensor(out=y[:], in0=y[:], in1=beta_c[:], op=ADD)
                            nc.sync.dma_start(out=out_d[t * 128 : (t + 1) * 128, :], in_=y[:])

    nc.compile()
    return nc


def _get_nc():
    if "nc" not in _CACHE:
        _CACHE["nc"] = _build()
    return _CACHE["nc"]


def kernel(X, Wq, bq, Wk, bk, Wv, bv, Wo, bo, gamma, beta):
    if os.environ.get("BASS_TRACE"):
        _install_ntff_hook()
    import ml_dtypes

    from concourse.bass_utils import run_bass_kernel_spmd

    bfdt = ml_dtypes.bfloat16
    f32 = np.float32
    X = np.ascontiguousarray(np.asarray(X, dtype=f32))
    wqt = np.ascontiguousarray(np.asarray(Wq, f32).T.astype(bfdt))
    wkt = np.ascontiguousarray(np.asarray(Wk, f32).T.astype(bfdt))
    wvt = np.ascontiguousarray(np.asarray(Wv, f32).T.astype(bfdt))
    wot = np.ascontiguousarray(np.asarray(Wo, f32).T.astype(bfdt))
    bqt = np.ascontiguousarray(np.asarray(bq, f32).reshape(8, 128).T)
    bkt = np.ascontiguousarray(np.asarray(bk, f32).reshape(8, 128).T)
    bv_ = np.ascontiguousarray(np.asarray(bv, f32))
    bo_ = np.ascontiguousarray(np.asarray(bo, f32))
    gamma_ = np.ascontiguousarray(np.asarray(gamma, f32))
    beta_ = np.ascontiguousarray(np.asarray(beta, f32))
    sel = np.zeros((16, 8, 128), f32)
    for m in range(8):
        sel[2 * m, m, 0:64] = 1.0
        sel[2 * m + 1, m, 64:128] = 1.0
    sel = sel.astype(bfdt)

    in_maps = []
    for c in range(NCORES):
        b, half = c // 2, c % 2
        Xb = X[b]
        q_rows = Xb[half * SQ : (half + 1) * SQ]
        o_rows = Xb[(1 - half) * SQ : (2 - half) * SQ]
        # queries-first token order (key order is permutation-invariant)
        xt = np.ascontiguousarray(np.concatenate([q_rows, o_rows], axis=0).T.astype(bfdt))
        in_maps.append(
            {
                "xt": xt,
                "xq": np.ascontiguousarray(q_rows),
                "wqt": wqt,
                "wkt": wkt,
                "wvt": wvt,
                "wot": wot,
                "bqt": bqt,
                "bkt": bkt,
                "bv": bv_,
                "bo": bo_,
                "gamma": gamma_,
                "beta": beta_,
                "sel": sel,
            }
        )

    nc = _get_nc()
    res = run_bass_kernel_spmd(nc, in_maps, core_ids=list(range(NCORES)))
    if res.exec_time_ns is not None:
        print(f"HW exec time: {res.exec_time_ns} ns")

    out = np.empty((B, S, D), np.float32)
    for c in range(NCORES):
        b, half = c // 2, c % 2
        out[b, half * SQ : (half + 1) * SQ] = res.results[c]["out"]
    return out


# revision 6
# speedup vs baseline: 1.0527x; 1.0527x over previous
"""Trainium2 Bass kernel for nn_Attention (B=4, S=2048, D=1024, H=16) on 8 NeuronCores.

Sharding: data-parallel over (batch, sequence-half) -> 8 shards, one per core.
Each core computes attention for 1024 query tokens of one batch element:
  - K/V projections over the full 2048-token sequence of its batch element
    (duplicated across the 2 cores sharing a batch element -- cheaper than
    communicating K/V, so the kernel needs no collectives),
  - Q projection for its 1024 queries,
  - per-head S^T = K_h @ Q_h^T, softmax over keys via exp + ones-column
    denominator folded into the A@V matmul,
  - output projection + residual + LayerNorm on its 1024 tokens.

Pipeline: V projection runs first, then per head-pair m the Q/K projections for
that pair interleave with QK^T + exp + A@V of the previous pair, so the ScalarE
exp stream (the phase-2 bottleneck) overlaps TensorE projection work instead of
serializing after it. Q^T/K^T slices are streamed per head-pair. LayerNorm's
gamma/beta affine runs on the otherwise-idle GpSimd engine.

All matmuls are bf16 inputs with fp32 PSUM accumulation; softmax statistics,
residual and LayerNorm stay fp32.
"""

import os
import sys

sys.path.insert(0, "/opt/trn_rl_repo")

import numpy as np

B, S, D, H = 4, 2048, 1024, 16
HD = D // H  # 64
SQ = S // 2  # queries per core
NCORES = 8
EPS = 1e-12

_CACHE = {}


def _install_ntff_hook():
    """Register the axon NTFF profile hook that bass_utils looks up via
    antenv.axon_hooks (absent from the image's antenv stub)."""
    import contextlib
    import ctypes
    import types

    so_path = "/opt/axon/libaxon_pjrt.so"
    if "antenv.axon_hooks" in sys.modules:
        return
    try:
        lib = ctypes.CDLL(so_path)
    except OSError:
        return
    if not hasattr(lib, "axon_start_nrt_profile"):
        return
    lib.axon_start_nrt_profile.argtypes = [ctypes.POINTER(ctypes.c_int64), ctypes.c_size_t]
    lib.axon_start_nrt_profile.restype = ctypes.c_int64
    lib.axon_stop_nrt_profile.argtypes = [ctypes.c_char_p]
    lib.axon_stop_nrt_profile.restype = ctypes.c_int64

    @contextlib.contextmanager
    def _hook(output_dir, device_ids):
        import jax

        jax.devices()
        if device_ids:
            ids = (ctypes.c_int64 * len(device_ids))(*device_ids)
            rc = lib.axon_start_nrt_profile(ids, len(device_ids))
        else:
            rc = lib.axon_start_nrt_profile(None, 0)
        if rc != 0:
            raise RuntimeError(f"axon_start_nrt_profile rc={rc}")
        try:
            yield
        finally:
            n = lib.axon_stop_nrt_profile(str(output_dir).encode())
            if n < 0:
                raise RuntimeError(f"axon_stop_nrt_profile rc={n}")

    m = types.ModuleType("antenv.axon_hooks")
    m.get_axon_ntff_profile_hook = lambda: _hook
    m.set_axon_ntff_profile_hook = lambda h: None
    sys.modules["antenv.axon_hooks"] = m


def _build():
    import concourse.bass as bass
    import concourse.tile as tile
    from concourse import bacc, mybir

    f32 = mybir.dt.float32
    bf16 = mybir.dt.bfloat16
    ADD = mybir.AluOpType.add
    MULT = mybir.AluOpType.mult
    SUB = mybir.AluOpType.subtract
    Exp = mybir.ActivationFunctionType.Exp
    Sqrt = mybir.ActivationFunctionType.Sqrt

    nc = bacc.Bacc("TRN2")

    xt_d = nc.dram_tensor("xt", [D, S], bf16, kind="ExternalInput")
    xq_d = nc.dram_tensor("xq", [SQ, D], f32, kind="ExternalInput")
    wq_d = nc.dram_tensor("wqt", [D, D], bf16, kind="ExternalInput")
    wk_d = nc.dram_tensor("wkt", [D, D], bf16, kind="ExternalInput")
    wv_d = nc.dram_tensor("wvt", [D, D], bf16, kind="ExternalInput")
    wo_d = nc.dram_tensor("wot", [D, D], bf16, kind="ExternalInput")
    bq_d = nc.dram_tensor("bqt", [128, 8], f32, kind="ExternalInput")
    bk_d = nc.dram_tensor("bkt", [128, 8], f32, kind="ExternalInput")
    bv_d = nc.dram_tensor("bv", [D], f32, kind="ExternalInput")
    bo_d = nc.dram_tensor("bo", [D], f32, kind="ExternalInput")
    gamma_d = nc.dram_tensor("gamma", [D], f32, kind="ExternalInput")
    beta_d = nc.dram_tensor("beta", [D], f32, kind="ExternalInput")
    sel2_d = nc.dram_tensor("sel2", [2, 128], bf16, kind="ExternalInput")
    out_d = nc.dram_tensor("out", [SQ, D], f32, kind="ExternalOutput")

    def bcast_ap(handle):
        ap = handle[:]
        return bass.AP(tensor=ap.tensor, offset=ap.offset, ap=[[0, 128], ap.ap[0]])

    with tile.TileContext(nc) as tc:
        with (
            tc.tile_pool(name="const", bufs=1) as constp,
            tc.tile_pool(name="v", bufs=1) as vp,
            tc.tile_pool(name="ot", bufs=1) as otp,
            tc.tile_pool(name="xt", bufs=1) as xtp,
        ):
            # --- constants ---
            bq_c = constp.tile([128, 8], f32, tag="bq")
            bk_c = constp.tile([128, 8], f32, tag="bk")
            bv_c = constp.tile([128, D], f32, tag="bv")
            bo_c = constp.tile([128, D], f32, tag="bo")
            gamma_c = constp.tile([128, D], f32, tag="gamma")
            beta_c = constp.tile([128, D], f32, tag="beta")
            eps_c = constp.tile([128, 1], f32, tag="eps")
            sel2_c = constp.tile([2, 128], bf16, tag="sel2")
            nc.sync.dma_start(out=bq_c[:], in_=bq_d[:])
            nc.sync.dma_start(out=bk_c[:], in_=bk_d[:])
            nc.gpsimd.dma_start(out=bv_c[:], in_=bcast_ap(bv_d))
            nc.gpsimd.dma_start(out=bo_c[:], in_=bcast_ap(bo_d))
            nc.gpsimd.dma_start(out=gamma_c[:], in_=bcast_ap(gamma_d))
            nc.gpsimd.dma_start(out=beta_c[:], in_=bcast_ap(beta_d))
            nc.sync.dma_start(out=sel2_c[:], in_=sel2_d[:])
            nc.vector.memset(eps_c[:], EPS)

            # --- persistent activations ---
            v = vp.tile([128, 16, H, HD + 1], bf16, tag="v")  # V + ones col
            ot = otp.tile([128, 8, SQ], bf16, tag="ot")       # O^T
            xt = xtp.tile([128, 8, S], bf16, tag="xt")

            nc.vector.memset(v[:, :, :, HD : HD + 1], 1.0)
            for r in range(8):
                nc.sync.dma_start(out=xt[:, r, :], in_=xt_d[r * 128 : (r + 1) * 128, :])

            with (
                tc.tile_pool(name="wvr", bufs=1) as wvrp,
                tc.tile_pool(name="qkw", bufs=2) as qkwp,
                tc.tile_pool(name="qts", bufs=2) as qtsp,
                tc.tile_pool(name="kts", bufs=2) as ktsp,
                tc.tile_pool(name="st", bufs=6) as stp,
                tc.tile_pool(name="stage", bufs=2) as stagep,
                tc.tile_pool(name="dens", bufs=1) as densp,
                tc.tile_pool(name="ps1", bufs=2, space="PSUM") as ps1,
                tc.tile_pool(name="sp", bufs=2, space="PSUM") as spp,
                tc.tile_pool(name="av", bufs=1, space="PSUM") as avp,
            ):
                # ---------- V projection (first: A@V consumes all of it) ----------
                wv_r = wvrp.tile([128, 8, D], bf16, tag="wvr")
                for k in range(8):
                    nc.sync.dma_start(
                        out=wv_r[:, k, :], in_=wv_d[k * 128 : (k + 1) * 128, :]
                    )
                for tc_i in range(16):
                    for dg in range(2):
                        psv = ps1.tile([128, 512], f32, tag="ps", name="psv")
                        for k in range(8):
                            nc.tensor.matmul(
                                out=psv[:],
                                lhsT=xt[:, k, tc_i * 128 : (tc_i + 1) * 128],
                                rhs=wv_r[:, k, dg * 512 : (dg + 1) * 512],
                                start=(k == 0),
                                stop=(k == 7),
                            )
                        nc.vector.tensor_tensor(
                            out=v[:, tc_i, dg * 8 : (dg + 1) * 8, 0:HD],
                            in0=psv[:].rearrange("p (h d) -> p h d", d=HD),
                            in1=bv_c[:, dg * 512 : (dg + 1) * 512].rearrange(
                                "p (h d) -> p h d", d=HD
                            ),
                            op=ADD,
                        )

                # ---------- interleaved: per head-pair m ----------
                # emit block m:  Q(m), K(m) projections; QK^T+exp heads 2m,2m+1;
                # then A@V + evac + normalize for the previous pair (lag 1).
                def proj_block(m):
                    wq_m = qkwp.tile([128, 8, 128], bf16, tag="qkw", name="wq_m")
                    nc.sync.dma_start(
                        out=wq_m[:],
                        in_=wq_d[:, m * 128 : (m + 1) * 128].rearrange(
                            "(k p) c -> p k c", p=128
                        ),
                    )
                    qt_m = qtsp.tile([128, SQ], bf16, tag="qts", name="qt_m")
                    for tg in range(2):
                        psq = ps1.tile([128, 512], f32, tag="ps", name="psq")
                        for k in range(8):
                            nc.tensor.matmul(
                                out=psq[:],
                                lhsT=wq_m[:, k, :],
                                rhs=xt[:, k, tg * 512 : (tg + 1) * 512],
                                start=(k == 0),
                                stop=(k == 7),
                            )
                        nc.vector.tensor_scalar(
                            out=qt_m[:, tg * 512 : (tg + 1) * 512],
                            in0=psq[:],
                            scalar1=bq_c[:, m : m + 1],
                            scalar2=None,
                            op0=ADD,
                        )
                    wk_m = qkwp.tile([128, 8, 128], bf16, tag="qkw", name="wk_m")
                    nc.sync.dma_start(
                        out=wk_m[:],
                        in_=wk_d[:, m * 128 : (m + 1) * 128].rearrange(
                            "(k p) c -> p k c", p=128
                        ),
                    )
                    kt_m = ktsp.tile([128, S], bf16, tag="kts", name="kt_m")
                    for tg in range(4):
                        psk = ps1.tile([128, 512], f32, tag="ps", name="psk")
                        for k in range(8):
                            nc.tensor.matmul(
                                out=psk[:],
                                lhsT=wk_m[:, k, :],
                                rhs=xt[:, k, tg * 512 : (tg + 1) * 512],
                                start=(k == 0),
                                stop=(k == 7),
                            )
                        nc.vector.tensor_scalar(
                            out=kt_m[:, tg * 512 : (tg + 1) * 512],
                            in0=psk[:],
                            scalar1=bk_c[:, m : m + 1],
                            scalar2=None,
                            op0=ADD,
                        )
                    return qt_m, kt_m

                def qk_exp_block(m, qt_m, kt_m):
                    st_pair = []
                    for hh in range(2):  # heads 2m, 2m+1
                        p0 = hh * 64
                        st_tiles = [
                            stp.tile([128, 4, SQ], bf16, tag="st", name="st")
                            for _ in range(4)
                        ]
                        for kc in range(16):
                            sp = spp.tile([128, 1024], f32, tag="sp", name="sp")
                            for qh in range(2):
                                nc.tensor.matmul(
                                    out=sp[:, qh * 512 : (qh + 1) * 512],
                                    lhsT=kt_m[p0 : p0 + 64, kc * 128 : (kc + 1) * 128],
                                    rhs=qt_m[p0 : p0 + 64, qh * 512 : (qh + 1) * 512],
                                    start=True,
                                    stop=True,
                                )
                            nc.scalar.activation(
                                out=st_tiles[kc // 4][:, kc % 4, :],
                                in_=sp[:],
                                func=Exp,
                                scale=float(1.0 / np.sqrt(HD)),
                            )
                        st_pair.append(st_tiles)
                    return st_pair

                def av_block(m, st_pair):
                    den_m = densp.tile([2, SQ], f32, tag="den", name="den_m")
                    for hh in range(2):
                        h = 2 * m + hh
                        st_tiles = st_pair[hh]
                        av = avp.tile([128, 1024], f32, tag="av", name="av")
                        for qh in range(2):
                            for kc in range(16):
                                nc.tensor.matmul(
                                    out=av[0:65, qh * 512 : (qh + 1) * 512],
                                    lhsT=v[:, kc, h, :],
                                    rhs=st_tiles[kc // 4][
                                        :, kc % 4, qh * 512 : (qh + 1) * 512
                                    ],
                                    start=(kc == 0),
                                    stop=(kc == 15),
                                )
                        stg_d = stagep.tile([65, SQ], f32, tag="stg_d", name="stg_d")
                        nc.vector.tensor_copy(out=stg_d[64:65, :], in_=av[64:65, :])
                        nc.sync.dma_start(out=den_m[hh : hh + 1, :], in_=stg_d[64:65, :])
                        if hh == 0:
                            nc.vector.tensor_copy(out=ot[0:64, m, :], in_=av[0:64, :])
                        else:
                            stg_o = stagep.tile([64, SQ], bf16, tag="stg_o", name="stg_o")
                            nc.vector.tensor_copy(out=stg_o[:, :], in_=av[0:64, :])
                            nc.sync.dma_start(out=ot[64:128, m, :], in_=stg_o[:, :])
                    # normalize: ot[:, m, :] *= 1/den broadcast over the head dims
                    rc_f = densp.tile([2, SQ], f32, tag="rcf", name="rc_f")
                    rc_b = densp.tile([2, SQ], bf16, tag="rcb", name="rc_b")
                    nc.vector.reciprocal(out=rc_f[:], in_=den_m[:])
                    nc.vector.tensor_copy(out=rc_b[:], in_=rc_f[:])
                    bc = avp.tile([128, 1024], f32, tag="av", name="bc")
                    for g in range(2):
                        nc.tensor.matmul(
                            out=bc[:, g * 512 : (g + 1) * 512],
                            lhsT=sel2_c[:],
                            rhs=rc_b[:, g * 512 : (g + 1) * 512],
                            start=True,
                            stop=True,
                        )
                    nc.vector.tensor_tensor(out=ot[:, m, :], in0=ot[:, m, :], in1=bc[:], op=MULT)

                prev = None
                for m in range(8):
                    qt_m, kt_m = proj_block(m)
                    st_pair = qk_exp_block(m, qt_m, kt_m)
                    if prev is not None:
                        av_block(prev[0], prev[1])
                    prev = (m, st_pair)
                av_block(prev[0], prev[1])

            # ========== phase 3: output projection + residual + LN ==========
            with (
                tc.tile_pool(name="wo", bufs=3) as wop,
                tc.tile_pool(name="xqp", bufs=3) as xqp,
                tc.tile_pool(name="xqbo", bufs=3) as xqbop,
                tc.tile_pool(name="y", bufs=3) as yp,
                tc.tile_pool(name="y2", bufs=3) as y2p,
                tc.tile_pool(name="stats", bufs=4) as statp,
                tc.tile_pool(name="ps3", bufs=4, space="PSUM") as ps3,
            ):
                for tg in range(2):
                    ps = [ps3.tile([128, 1024], f32, tag="ps", name="ps3") for _ in range(4)]
                    for k in range(8):
                        wt = wop.tile([128, D], bf16, tag="wo")
                        nc.sync.dma_start(out=wt[:], in_=wo_d[k * 128 : (k + 1) * 128, :])
                        for i in range(4):
                            t = tg * 4 + i
                            for g in range(2):
                                nc.tensor.matmul(
                                    out=ps[i][:, g * 512 : (g + 1) * 512],
                                    lhsT=ot[:, k, t * 128 : (t + 1) * 128],
                                    rhs=wt[:, g * 512 : (g + 1) * 512],
                                    start=(k == 0),
                                    stop=(k == 7),
                                )
                    for i in range(4):
                        t = tg * 4 + i
                        xq_t = xqp.tile([128, D], f32, tag="xq")
                        nc.sync.dma_start(out=xq_t[:], in_=xq_d[t * 128 : (t + 1) * 128, :])
                        xqbo = xqbop.tile([128, D], f32, tag="xqbo")
                        nc.gpsimd.tensor_tensor(out=xqbo[:], in0=xq_t[:], in1=bo_c[:], op=ADD)
                        y = yp.tile([128, D], f32, tag="y")
                        nc.vector.tensor_tensor(out=y[:], in0=ps[i][:], in1=xqbo[:], op=ADD)
                        stats = statp.tile([128, 2, 6], f32, tag="stats")
                        mv = statp.tile([128, 2], f32, tag="mv")
                        nc.vector.bn_stats(out=stats[:, 0, :], in_=y[:, 0:512])
                        nc.vector.bn_stats(out=stats[:, 1, :], in_=y[:, 512:1024])
                        nc.vector.bn_aggr(out=mv[:], in_=stats[:])
                        nc.scalar.activation(
                            out=mv[:, 1:2], in_=mv[:, 1:2], func=Sqrt, bias=eps_c[:, 0:1]
                        )
                        nc.vector.reciprocal(out=mv[:, 1:2], in_=mv[:, 1:2])
                        nc.vector.tensor_scalar(
                            out=y[:],
                            in0=y[:],
                            scalar1=mv[:, 0:1],
                            scalar2=mv[:, 1:2],
                            op0=SUB,
                            op1=MULT,
                        )
                        y2 = y2p.tile([128, D], f32, tag="y2")
                        nc.gpsimd.tensor_tensor(out=y2[:], in0=y[:], in1=gamma_c[:], op=MULT)
                        nc.gpsimd.tensor_tensor(out=y2[:], in0=y2[:], in1=beta_c[:], op=ADD)
                        nc.sync.dma_start(out=out_d[t * 128 : (t + 1) * 128, :], in_=y2[:])

    nc.compile()
    return nc


def _get_nc():
    if "nc" not in _CACHE:
        _CACHE["nc"] = _build()
    return _CACHE["nc"]


def kernel(X, Wq, bq, Wk, bk, Wv, bv, Wo, bo, gamma, beta):
    if os.environ.get("BASS_TRACE"):
        _install_ntff_hook()
    import ml_dtypes

    from concourse.bass_utils import run_bass_kernel_spmd

    bfdt = ml_dtypes.bfloat16
    f32 = np.float32
    X = np.ascontiguousarray(np.asarray(X, dtype=f32))
    wqt = np.ascontiguousarray(np.asarray(Wq, f32).T.astype(bfdt))
    wkt = np.ascontiguousarray(np.asarray(Wk, f32).T.astype(bfdt))
    wvt = np.ascontiguousarray(np.asarray(Wv, f32).T.astype(bfdt))
    wot = np.ascontiguousarray(np.asarray(Wo, f32).T.astype(bfdt))
    bqt = np.ascontiguousarray(np.asarray(bq, f32).reshape(8, 128).T)
    bkt = np.ascontiguousarray(np.asarray(bk, f32).reshape(8, 128).T)
    bv_ = np.ascontiguousarray(np.asarray(bv, f32))
    bo_ = np.ascontiguousarray(np.asarray(bo, f32))
    gamma_ = np.ascontiguousarray(np.asarray(gamma, f32))
    beta_ = np.ascontiguousarray(np.asarray(beta, f32))
    sel2 = np.zeros((2, 128), f32)
    sel2[0, 0:64] = 1.0
    sel2[1, 64:128] = 1.0
    sel2 = sel2.astype(bfdt)

    in_maps = []
    for c in range(NCORES):
        b, half = c // 2, c % 2
        Xb = X[b]
        q_rows = Xb[half * SQ : (half + 1) * SQ]
        o_rows = Xb[(1 - half) * SQ : (2 - half) * SQ]
        # queries-first token order (key order is permutation-invariant)
        xt = np.ascontiguousarray(np.concatenate([q_rows, o_rows], axis=0).T.astype(bfdt))
        in_maps.append(
            {
                "xt": xt,
                "xq": np.ascontiguousarray(q_rows),
                "wqt": wqt,
                "wkt": wkt,
                "wvt": wvt,
                "wot": wot,
                "bqt": bqt,
                "bkt": bkt,
                "bv": bv_,
                "bo": bo_,
                "gamma": gamma_,
                "beta": beta_,
                "sel2": sel2,
            }
        )

    nc = _get_nc()
    res = run_bass_kernel_spmd(nc, in_maps, core_ids=list(range(NCORES)))
    if res.exec_time_ns is not None:
        print(f"HW exec time: {res.exec_time_ns} ns")

    out = np.empty((B, S, D), np.float32)
    for c in range(NCORES):
        b, half = c // 2, c % 2
        out[b, half * SQ : (half + 1) * SQ] = res.results[c]["out"]
    return out
